# revision 24
# baseline (speedup 1.0000x reference)
"""Multi-head attention (B=2, S=2048, E=1024, H=16, D=64) on 8 TRN2 NeuronCores.

Sharding: tensor-parallel over heads (2 heads/core) for QKV projections and
attention; an on-device AllToAll reshards the attention output so each core
owns 512 rows; row-parallel output projection; host concatenates the row
slices. Inputs are host-cast to bf16 and x is host-transposed (the
contraction dim must sit on SBUF partitions); all matmul accumulation is
fp32 on-chip.

v2: the attention phase is emitted as a fine-grained interleave so the PE
instruction queue never sits idle long enough for the HAM clock-gate to
re-throttle (the v1 trace showed the PE at K=4/8 for 87% of the kernel).
Per (batch, q-block) unit there are 8 groups of 2 key-column tiles; each
group emits 4 score matmuls (two heads on disjoint 64-row PE groups, which
the PE runs concurrently), 2 ACT exp calls (one per head, N=1024/lane),
4 PV matmuls of the previous unit (head 0's chain during groups 0-3,
head 1's during 4-7 -> a single PV PSUM bank suffices), plus filler:
batch-1 QKV projection chains spread 4 MMs/group over units 0-2, batch-1
V-transposes in unit 3, and pace-padding dummy matmuls in units 4-7.
PSUM budget: 3x2-bank score tiles + 1 PV bank + 1 aux bank = 8.

Normalization: one batched reciprocal per unit ([2,512] covers both heads),
PV rows staged to SBUF bf16 at chain end, softmax scale applied there;
the AllToAll and row-parallel out-projection are unchanged from v1.
"""

import sys

if "/opt/trn_rl_repo" not in sys.path:
    sys.path.insert(0, "/opt/trn_rl_repo")

from contextlib import ExitStack

import numpy as np

import concourse.bacc as bacc
import concourse.mybir as mybir
import concourse.tile as tile
from concourse.masks import make_identity

F32 = mybir.dt.float32
BF16 = mybir.dt.bfloat16
AF = mybir.ActivationFunctionType

_CACHE = {}


def build_kernel(B=2, S=2048, E=1024, H=16, D=64, N_CORES=8):
    HL = H // N_CORES
    HIDL = HL * D
    R = B * S
    RL = R // N_CORES
    EC = E // 128
    S128 = S // 128
    QB = 512
    NQB = S // QB
    RT = R // 128
    NG = R // QB
    assert HIDL == 128 and D == 64 and QB == RL
    assert NG == N_CORES and S % QB == 0

    nc = bacc.Bacc("TRN2", target_bir_lowering=False, debug=False,
                   num_devices=N_CORES)

    # host-packed layouts: every load below is a single fully-contiguous DMA
    xt_d = nc.dram_tensor("xt", [B * EC * 128, S], BF16, kind="ExternalInput")
    wq_d = nc.dram_tensor("wq", [128, EC * HIDL], BF16, kind="ExternalInput")
    wk_d = nc.dram_tensor("wk", [128, EC * HIDL], BF16, kind="ExternalInput")
    wv_d = nc.dram_tensor("wv", [128, EC * HIDL], BF16, kind="ExternalInput")
    wo_d = nc.dram_tensor("wo", [128, EC * E], BF16, kind="ExternalInput")
    bqkv_d = nc.dram_tensor("bqkv", [HIDL, 3], F32, kind="ExternalInput")
    bo_d = nc.dram_tensor("bo", [1, E], BF16, kind="ExternalInput")
    out_d = nc.dram_tensor("out", [RL, E], F32, kind="ExternalOutput")

    with tile.TileContext(nc) as tc, ExitStack() as ctx:
        const = ctx.enter_context(tc.tile_pool(name="const", bufs=1))
        big = ctx.enter_context(tc.tile_pool(name="big", bufs=1))
        stage = ctx.enter_context(tc.tile_pool(name="stage", bufs=4))
        dram = ctx.enter_context(tc.tile_pool(name="dram", bufs=1, space="DRAM"))

        # ---- weights (one coalesced DMA per matrix) + x^T loads first:
        # batch-0 halves ahead of batch-1 so the batch-0 projection chains
        # can ride the chunk arrivals ----
        xT = big.tile([128, B, EC, S], BF16)
        w_tiles = {}
        wk_sb = const.tile([128, EC, HIDL], BF16, name="wk_all")
        nc.scalar.dma_start(out=wk_sb[:], in_=wk_d[:])
        for i in range(EC):
            w_tiles[("wk", i)] = wk_sb[:, i, :]
        bqkv_sb = const.tile([HIDL, 3], F32, name="bqkv_sb")
        nc.sync.dma_start(out=bqkv_sb[:], in_=bqkv_d[:])
        b_tiles = {"bq": bqkv_sb[:, 0:1], "bk": bqkv_sb[:, 1:2],
                   "bv": bqkv_sb[:, 2:3]}
        # batch-0 x chunks next (chains are paced by them); wq/wv ride the
        # scalar queue behind the b0-even chunks, in time for chain set 2
        for i in range(EC):
            eng = nc.scalar if i % 2 == 0 else nc.sync
            eng.dma_start(out=xT[:, 0, i, :], in_=xt_d[128 * i:128 * (i + 1), :])
        for wname, wd in (("wq", wq_d), ("wv", wv_d)):
            t = const.tile([128, EC, HIDL], BF16, name=f"{wname}_all")
            nc.scalar.dma_start(out=t[:], in_=wd[:])
            for i in range(EC):
                w_tiles[(wname, i)] = t[:, i, :]
        for i in range(EC):
            eng = nc.scalar if i % 2 == 0 else nc.sync
            eng.dma_start(out=xT[:, 1, i, :],
                          in_=xt_d[(EC + i) * 128:(EC + i + 1) * 128, :])
        # (wo / bo are loaded post-attention, into the space the score
        # pipeline frees up; their DMAs overlap the AllToAll window)

        # constants (DVE work overlaps the input DMAs)
        ident = const.tile([128, 128], BF16)
        make_identity(nc, ident)
        ones_st = const.tile([1, 128], BF16)
        nc.vector.memset(ones_st, 1.0)
        dconst = const.tile([128, QB], BF16)
        nc.vector.memset(dconst, 1.0)

        # dummy collective #1: absorbs cross-core launch skew
        sync_sb = const.tile([128, 4], F32)
        nc.vector.memset(sync_sb, 1.0)
        sync_in = dram.tile([128, 4], F32)
        sync_out = dram.tile([128, 4], F32)
        nc.sync.dma_start(out=sync_in[:], in_=sync_sb[:])
        nc.gpsimd.collective_compute(
            "AllReduce", mybir.AluOpType.add,
            replica_groups=[list(range(N_CORES))],
            ins=[sync_in.opt()], outs=[sync_out.opt()])

        # ---- projection / attention state ----
        QT = big.tile([128, R], BF16)
        KT = big.tile([128, R], BF16)
        VT = big.tile([128, R], BF16)
        # row pitch 80 elems = 160 B keeps every (hl, kt) slice 32 B-aligned
        # (the xbar DMA-transpose writes require it); cols D+1..79 are unused
        VP = 80
        Vext = big.tile([128, HL, RT, VP], BF16)
        ATn = big.tile([128, NG, QB], BF16)

        rp = ctx.enter_context(tc.tile_pool(name="rp", bufs=2))
        att_stack = ExitStack()
        att = att_stack.enter_context(tc.tile_pool(name="att", bufs=4))
        sc_psum = att_stack.enter_context(
            tc.tile_pool(name="sc_psum", bufs=3, space="PSUM"))
        pv_psum = att_stack.enter_context(
            tc.tile_pool(name="pv_psum", bufs=1, space="PSUM"))
        aux_psum = att_stack.enter_context(
            tc.tile_pool(name="aux_psum", bufs=1, space="PSUM"))

        # PE pace-padding: dense dummy matmuls with no real consumers keep
        # the HAM clock-gate warm across windows where no real PE work is
        # runnable. All dummies WAW-chain into the aux bank; one byte is
        # DMA'd out at the end so the chain is kept.
        wup_sink = dram.tile([1, 4], BF16)
        wup_sb = const.tile([1, 4], BF16)

        def dummy_mms(n, nf=QB):
            wps = aux_psum.tile([128, QB], F32, tag="aux", name="wps")
            for _ in range(n):
                nc.tensor.matmul(wps[:, 0:nf], ident[:], dconst[:, 0:nf],
                                 start=True, stop=True)

        def dummy_flush():
            wps = aux_psum.tile([128, QB], F32, tag="aux", name="wps")
            nc.tensor.matmul(wps[:, 0:4], ident[:], dconst[:, 0:4],
                             start=True, stop=True)
            nc.vector.tensor_copy(out=wup_sb[:], in_=wps[0:1, 0:4])

        def dummy_sc(n, nf=256):
            """Pre-attention dummies live in the (then unused) score pool."""
            wps = sc_psum.tile([128, 2, QB], F32, tag="sc", name="wdum")
            for _ in range(n):
                nc.tensor.matmul(wps[:, 0, 0:nf], ident[:], dconst[:, 0:nf],
                                 start=True, stop=True)

        # warm the PE across the start of the x-DMA window
        dummy_sc(12)

        # ---- QKV projection helpers ----
        def proj_rb(wname, bname, out_t, rb):
            """Full projection chain for one 512-row block (emit at once)."""
            ps = aux_psum.tile([128, QB], F32, tag="aux", name="qkv_ps")
            for i in range(EC):
                nc.tensor.matmul(ps[:], w_tiles[(wname, i)],
                                 xT[:, rb // NQB, i,
                                    QB * (rb % NQB):QB * (rb % NQB + 1)],
                                 start=(i == 0), stop=(i == EC - 1))
            nc.vector.tensor_scalar_add(
                out=out_t[:, QB * rb:QB * (rb + 1)], in0=ps[:],
                scalar1=b_tiles[bname][:])

        # spread variant: emitted a few MMs at a time from the unit loop
        class ProjFeed:
            def __init__(self, jobs, per_group=4):
                self.jobs = list(jobs)  # (wname, bname, out_t, rb)
                self.ji = 0
                self.mi = 0
                self.ps = None
                self.per_group = per_group

            def emit(self, n):
                for _ in range(n):
                    if self.ji >= len(self.jobs):
                        return
                    wname, bname, out_t, rb = self.jobs[self.ji]
                    if self.mi == 0:
                        self.ps = aux_psum.tile([128, QB], F32, tag="aux",
                                                name="qkv_ps")
                    i = self.mi
                    nc.tensor.matmul(self.ps[:], w_tiles[(wname, i)],
                                     xT[:, rb // NQB, i,
                                        QB * (rb % NQB):QB * (rb % NQB + 1)],
                                     start=(i == 0), stop=(i == EC - 1))
                    self.mi += 1
                    if self.mi == EC:
                        nc.vector.tensor_scalar_add(
                            out=out_t[:, QB * rb:QB * (rb + 1)],
                            in0=self.ps[:], scalar1=b_tiles[bname][:])
                        self.mi = 0
                        self.ji += 1

            def done(self):
                return self.ji >= len(self.jobs)

        # softmax-denominator ones columns, once for all key tiles
        nc.vector.memset(Vext[:, :, :, D:D + 1], 1.0)

        def chain_bank(which):
            if which == 0:
                return aux_psum.tile([128, QB], F32, tag="aux", name="cb")
            if which == 1:
                return pv_psum.tile([128, QB], F32, tag="pv", name="cb")
            return sc_psum.tile([128, 2, QB], F32, tag="sc", name="cb")[:, 0, :]

        def chain_set(jobs, pad=0):
            """Run up to 4 projection chains MM-interleaved over 4 distinct
            PSUM banks (aux + pv + 2 score-pool banks; pre-attention these
            are all free), so no chain waits another's DVE evacuation."""
            pss = [chain_bank(0 if c == 0 else (1 if c == 1 else 2))
                   for c in range(len(jobs))]
            for i in range(EC):
                for c, (wname, bname, out_t, rb) in enumerate(jobs):
                    nc.tensor.matmul(pss[c][:], w_tiles[(wname, i)],
                                     xT[:, rb // NQB, i,
                                        QB * (rb % NQB):QB * (rb % NQB + 1)],
                                     start=(i == 0), stop=(i == EC - 1))
                if pad:
                    dummy_sc(pad)
            for c, (wname, bname, out_t, rb) in enumerate(jobs):
                nc.vector.tensor_scalar_add(
                    out=out_t[:, QB * rb:QB * (rb + 1)], in0=pss[c][:],
                    scalar1=b_tiles[bname][:])

        # batch-0 projections: K first (scores need every key tile), then
        # Q block 0 + most of V; the first set is paced by the x-chunk
        # arrivals. Everything else becomes attention filler.
        chain_set([("wk", "bk", KT, rb) for rb in range(NQB)])
        chain_set([("wq", "bq", QT, 0)] + [("wv", "bv", VT, rb) for rb in range(3)])

        def vext_kt(kt, pool_tag="sc"):
            """V transpose via the PE into a currently-idle PSUM bank."""
            if pool_tag == "sc":
                ps = sc_psum.tile([128, 128], BF16, tag="sc", name="vtr_ps")
            elif pool_tag == "pv":
                ps = pv_psum.tile([128, 128], BF16, tag="pv", name="vtr_ps")
            else:
                ps = aux_psum.tile([128, 128], BF16, tag="aux", name="vtr_ps")
            nc.tensor.transpose(ps[:], VT[:, 128 * kt:128 * (kt + 1)], ident[:])
            for hl in range(HL):
                nc.vector.tensor_copy(out=Vext[:, hl, kt, 0:D],
                                      in_=ps[:, D * hl:D * (hl + 1)])

        # dummy collective #2: re-sync before the attention phase
        sync2_in = dram.tile([128, 4], BF16)
        sync2_out = dram.tile([128, 4], BF16)
        nc.sync.dma_start(out=sync2_in[:], in_=Vext[:, HL - 1, S128 - 1, 0:4])
        nc.gpsimd.collective_compute(
            "AllReduce", mybir.AluOpType.add,
            replica_groups=[list(range(N_CORES))],
            ins=[sync2_in.opt()], outs=[sync2_out.opt()])

        # ---- attention ----
        a2a_in = dram.tile([NG * HIDL, QB], BF16)
        a2a_out = dram.tile([NG * HIDL, QB], BF16)

        # filler plan: remaining projections spread 4 chain-MMs per group
        # over units 0-3 (aux bank holds exactly one open chain at a time);
        # batch-1 V-transposes go over the idle DMA engines (sync queue)
        # during units 2-3; units 4-7 pad with aux-bank dummies
        unit_feeds = {
            0: ProjFeed([("wv", "bv", VT, 3), ("wq", "bq", QT, 1),
                         ("wk", "bk", KT, 4), ("wk", "bk", KT, 5)]),
            1: ProjFeed([("wq", "bq", QT, 2), ("wq", "bq", QT, 3),
                         ("wk", "bk", KT, 6)], per_group=3),
            2: ProjFeed([("wk", "bk", KT, 7), ("wv", "bv", VT, 4),
                         ("wv", "bv", VT, 5)], per_group=3),
            3: ProjFeed([("wv", "bv", VT, 6), ("wv", "bv", VT, 7),
                         ("wq", "bq", QT, 4), ("wq", "bq", QT, 5)]),
            5: ProjFeed([("wq", "bq", QT, 6), ("wq", "bq", QT, 7)],
                        per_group=2),
        }


        units = [(b, qb) for b in range(B) for qb in range(NQB)]
        NGRP = S128 // 2  # 8 groups of 2 key tiles per unit

        def unit_scores_group(b, qb, g, Ebs):
            """4 score MMs (2 kc x 2 heads, head-paired) + 2 exp calls."""
            q0 = b * S + QB * qb
            pss = [sc_psum.tile([128, 2, QB], F32, tag="sc",
                                name=f"sc_ps{hl}") for hl in range(HL)]
            for j in range(2):
                kc = 2 * g + j
                for hl in range(HL):
                    hs = slice(64 * hl, 64 * (hl + 1))
                    nc.tensor.matmul(
                        pss[hl][:, j, :],
                        KT[hs, b * S + 128 * kc:b * S + 128 * (kc + 1)],
                        QT[hs, q0:q0 + QB], start=True, stop=True)
            for hl in range(HL):
                nc.scalar.activation(Ebs[hl][:, 2 * g:2 * g + 2, :],
                                     pss[hl][:], AF.Exp, scale=0.125)

        # PV pipeline state (one unit behind scores)
        pv_state = {}

        def pv_open(u_prev, Ebs_prev):
            b, qb = units[u_prev]
            pv_state.update(u=u_prev, b=b, qb=qb, Ebs=Ebs_prev,
                            g=(b * S + QB * qb) // QB,
                            kc=0, hl=0, pvT=None)

        def pv_emit(nmm):
            """Emit nmm PV matmuls of the open previous unit."""
            if not pv_state or pv_state.get("hl", 2) >= HL:
                return
            b, qb, Ebs = pv_state["b"], pv_state["qb"], pv_state["Ebs"]
            g = pv_state["g"]
            for _ in range(nmm):
                hl = pv_state["hl"]
                if hl >= HL:
                    return
                kc = pv_state["kc"]
                if kc == 0:
                    pv_state["pvT"] = pv_psum.tile([D + 1, QB], F32, tag="pv",
                                                   name="pvT")
                pvT = pv_state["pvT"]
                nc.tensor.matmul(
                    pvT[:], Vext[:, hl, b * S128 + kc, 0:D + 1],
                    Ebs[hl][:, kc, :],
                    start=(kc == 0), stop=(kc == S128 - 1))
                pv_state["kc"] += 1
                if pv_state["kc"] == S128:
                    # chain done: stage PV rows + denominator to SBUF
                    # (per-head staging tiles at partition base 0; the
                    # denominators land at partitions 0 / 32 -- engine APs
                    # need 32-aligned partition bases)
                    Ast = pv_state["Ast"][hl]
                    den = pv_state["den"]
                    nc.vector.tensor_copy(out=Ast[:], in_=pvT[0:D, :])
                    nc.vector.tensor_copy(out=den[32 * hl:32 * hl + 1, :],
                                          in_=pvT[D:D + 1, :])
                    pv_state["kc"] = 0
                    pv_state["hl"] += 1
                    if pv_state["hl"] == HL:
                        pv_finish(g, pv_state["Ast"], den)

        def pv_finish(g, Ast, den):
            """Batched reciprocal + broadcast + scale, then A2A staging."""
            rinv = rp.tile([33, QB], F32, tag="rinv", bufs=1, name="rinv")
            nc.vector.reciprocal(rinv[:], den[:])
            for hl in range(HL):
                hs = slice(64 * hl, 64 * (hl + 1))
                if hl == 0:
                    r_row = rinv[0:1, :]
                else:
                    # partition_broadcast sources partition 0 only; stage
                    # head 1's row down to partition 0 first
                    r1 = rp.tile([1, QB], F32, tag="r1", bufs=2, name="r1")
                    nc.vector.tensor_copy(out=r1[:], in_=rinv[32:33, :])
                    r_row = r1[:]
                r_sb = rp.tile([D, QB], F32, tag=f"r_sb{hl}", bufs=1,
                               name="r_sb")
                nc.gpsimd.partition_broadcast(r_sb[:], r_row)
                nc.vector.tensor_mul(out=ATn[hs, g, :], in0=Ast[hl][:],
                                     in1=r_sb[:])
            nc.sync.dma_start(out=a2a_in[HIDL * g:HIDL * (g + 1), :],
                              in_=ATn[:, g, :])

        prev = None
        late = {}
        for u, (b, qb) in enumerate(units):
            Ebs = [att.tile([128, S128, QB], BF16, tag="E", name="Eb")
                   for _ in range(HL)]
            if prev is not None:
                pv_open(u - 1, prev)
                pv_state["Ast"] = [stage.tile([D, QB], F32, tag=f"Ast{h}",
                                               bufs=1, name="Ast")
                                   for h in range(HL)]
                pv_state["den"] = rp.tile([33, QB], F32, tag="den", bufs=1,
                                          name="den")
                nc.vector.memset(pv_state["den"], 1.0)
            feed = unit_feeds.get(u)
            for g in range(NGRP):
                unit_scores_group(b, qb, g, Ebs)
                if prev is not None:
                    pv_emit(4)
                if feed is not None and not feed.done():
                    feed.emit(feed.per_group)
                if u == 4:
                    vext_kt(S128 + 2 * g, pool_tag="aux")
                    vext_kt(S128 + 2 * g + 1, pool_tag="aux")
                elif u == 6 or (u == 7 and g < 4):
                    dummy_mms(2)
                if u == 7 and g >= 4:
                    # late stream: the last unit's head-0 PV rides its own
                    # score groups (aux bank) so the post-attention drain
                    # only carries head 1
                    if g == 4:
                        late["pvT"] = aux_psum.tile([D + 1, QB], F32,
                                                    tag="aux", name="lateT")
                    for kc in range(4 * (g - 4), 4 * (g - 4) + 4):
                        nc.tensor.matmul(
                            late["pvT"][:], Vext[:, 0, S128 + kc, 0:D + 1],
                            Ebs[0][:, kc, :],
                            start=(kc == 0), stop=(kc == S128 - 1))
                if u == 0:
                    # batch-0 V transposes into the PV bank (idle in unit 0)
                    if g < 6:
                        vext_kt(2 * g, pool_tag="pv")
                        vext_kt(2 * g + 1, pool_tag="pv")
                    else:
                        vext_kt(12 + 2 * (g - 6), pool_tag="pv")
                        vext_kt(13 + 2 * (g - 6), pool_tag="pv")
            prev = Ebs

        # drain the last unit: head 0 already accumulated in the late
        # stream; only head 1's chain remains
        lb, lqb = units[-1]
        lg = (lb * S + QB * lqb) // QB
        Ast = [stage.tile([D, QB], F32, tag=f"Ast{h}", bufs=1, name="Ast")
               for h in range(HL)]
        den = rp.tile([33, QB], F32, tag="den", bufs=1, name="den")
        nc.vector.memset(den, 1.0)
        nc.vector.tensor_copy(out=Ast[0][:], in_=late["pvT"][0:D, :])
        nc.vector.tensor_copy(out=den[0:1, :], in_=late["pvT"][D:D + 1, :])
        pvT = pv_psum.tile([D + 1, QB], F32, tag="pv", name="pvT")
        for kc in range(S128):
            nc.tensor.matmul(pvT[:], Vext[:, 1, lb * S128 + kc, 0:D + 1],
                             prev[1][:, kc, :],
                             start=(kc == 0), stop=(kc == S128 - 1))
        nc.vector.tensor_copy(out=Ast[1][:], in_=pvT[0:D, :])
        nc.vector.tensor_copy(out=den[32:33, :], in_=pvT[D:D + 1, :])
        pv_finish(lg, Ast, den)

        nc.gpsimd.collective_compute(
            "AllToAll", mybir.AluOpType.bypass,
            replica_groups=[list(range(N_CORES))],
            ins=[a2a_in.opt()], outs=[a2a_out.opt()])

        # keep the PE warm across the AllToAll wait (anchored on the
        # last attention output so it runs inside that window)
        dummy_mms(180, 256)
        dummy_flush()
        nc.sync.dma_start(out=wup_sink[:], in_=wup_sb[:])
        att_stack.close()

        # ---- out projection ----
        post = ctx.enter_context(tc.tile_pool(name="post", bufs=1))
        bo_sb = post.tile([1, E], BF16)
        nc.scalar.dma_start(out=bo_sb[:], in_=bo_d[:])
        wo_sb = post.tile([128, EC, E], BF16, name="wo_all")
        nc.scalar.dma_start(out=wo_sb[:, 0:EC // 2, :],
                            in_=wo_d[:, 0:EC // 2 * E])
        nc.sync.dma_start(out=wo_sb[:, EC // 2:EC, :],
                          in_=wo_d[:, EC // 2 * E:EC * E])
        wo_tiles = [wo_sb[:, i, :] for i in range(EC)]
        AT = big.tile([128, EC, RL], BF16)
        for i in range(N_CORES):
            eng = nc.sync if i % 2 == 0 else nc.scalar
            eng.dma_start(out=AT[:, i, :],
                          in_=a2a_out[HIDL * i:HIDL * (i + 1), :])
        with tc.tile_pool(name="ph6_psum", bufs=1, space="PSUM") as ph6_psum:
            for qq in range(RL // 128):
                o_sb = post.tile([128, E], F32, tag="osb", bufs=2)
                pss = [ph6_psum.tile([128, QB], F32, tag=f"op{e_c}", bufs=2,
                                     name=f"op_ps{e_c}")
                       for e_c in range(E // QB)]
                for e_c in range(E // QB):
                    nc.tensor.matmul(pss[e_c][:], ones_st[:],
                                     bo_sb[:, QB * e_c:QB * (e_c + 1)],
                                     start=True, stop=False)
                for i in range(EC):
                    for e_c in range(E // QB):
                        nc.tensor.matmul(pss[e_c][:],
                                         AT[:, i, 128 * qq:128 * (qq + 1)],
                                         wo_tiles[i][:, QB * e_c:QB * (e_c + 1)],
                                         start=False, stop=(i == EC - 1))
                for e_c in range(E // QB):
                    nc.vector.tensor_copy(out=o_sb[:, QB * e_c:QB * (e_c + 1)],
                                          in_=pss[e_c][:])
                eng = nc.sync if qq % 2 == 0 else nc.scalar
                eng.dma_start(out=out_d[128 * qq:128 * (qq + 1), :],
                              in_=o_sb[:])

    nc.compile()
    return nc


def shard_inputs(x, Wq, bq, Wk, bk, Wv, bv, Wo, bo, N_CORES=8):
    """Host-side sharding + packing: full fp32 inputs -> per-core in_maps.

    Weight matrices are pre-packed so every device load is one contiguous
    DMA: W [E, HIDL] -> [128, EC*HIDL] with W_packed[p, i*HIDL+c] =
    W[128*i+p, c]; x -> batch-major transposed chunks [B*EC*128, S]."""
    import ml_dtypes
    bf16 = ml_dtypes.bfloat16
    B, S, E = x.shape
    R = B * S
    EC = E // 128
    HIDL = E // N_CORES

    def pack_w(W):
        # [E, C] -> [128, EC*C]
        C = W.shape[1]
        return np.ascontiguousarray(
            W.reshape(EC, 128, C).transpose(1, 0, 2).reshape(128, EC * C)
        ).astype(bf16)

    # x^T in batch-major chunk layout: [B, EC, 128, S] -> [B*EC*128, S]
    xt = np.ascontiguousarray(
        x.transpose(0, 2, 1).reshape(B, EC, 128, S).reshape(B * EC * 128, S)
    ).astype(bf16)
    wo = pack_w(Wo)
    bo_b = np.ascontiguousarray(bo.reshape(1, E)).astype(bf16)
    in_maps = []
    for c in range(N_CORES):
        cs = slice(HIDL * c, HIDL * (c + 1))
        bqkv = np.ascontiguousarray(
            np.stack([bq[cs], bk[cs], bv[cs]], axis=1)).astype(np.float32)
        in_maps.append({
            "xt": xt,
            "wq": pack_w(Wq[:, cs]),
            "wk": pack_w(Wk[:, cs]),
            "wv": pack_w(Wv[:, cs]),
            "wo": wo,
            "bqkv": bqkv,
            "bo": bo_b,
        })
    return in_maps


def kernel(x, Wq, bq, Wk, bk, Wv, bv, Wo, bo):
    from concourse.bass_utils import run_bass_kernel_spmd

    args = [np.asarray(a, dtype=np.float32) for a in
            (x, Wq, bq, Wk, bk, Wv, bv, Wo, bo)]
    if "nc" not in _CACHE:
        _CACHE["nc"] = build_kernel()
    nc = _CACHE["nc"]
    in_maps = shard_inputs(*args)
    res = run_bass_kernel_spmd(nc, in_maps, core_ids=list(range(8)))
    out = np.concatenate([res.results[i]["out"] for i in range(8)], axis=0)
    return out.reshape(2, 2048, 1024)
